# revision 1
# baseline (speedup 1.0000x reference)
"""CRF negative log-likelihood on 8 Trainium2 NeuronCores.

Strategy
--------
B=256, S=1024, T=128.  mean(log_Z - score) over the batch.

The log-partition forward recursion is rewritten in exp space:
    P_t = (expA^T P_{t-1}) * expE_t          (P in [T, batch] layout)
so each step is one PE matmul (stationary exp(transitions), 128x128) plus
one DVE elementwise multiply.  A constant per-step bias C0 = log(T)+0.5 is
folded into exp(emissions - C0) so the state magnitude stays bounded with
no runtime renormalisation (emissions are O(1); the per-step log-growth
concentrates tightly around C0, drift over 512 steps is a few units).

Sharding (8 cores): 4 batch groups of 64, x {forward, backward} halves of
the sequence (meet-in-the-middle halves the 1023-step serial chain).  The
backward recursion is exactly the forward program run on time-reversed
emissions with the transposed transition matrix, so all 8 cores run ONE
SPMD program:
    u_0 = exp(bvec) * e_0 ;  u_t = (M u_{t-1}) * e_t, t=1..511 ; chi = M u_511
with per-core (bvec, M) = (start, expA) fwd / (end, expA^T) bwd, and
    log Z_b = log(sum_j chi_fwd[j,b] chi_bwd[j,b]) + 1024*C0.

The gold path score is computed on-device with one-hot matmuls:
    G = sum_s E[s,:]^T O1[s,:]   (PE)  ->  emit part  = sum_j G[j,j]
    C = sum_s O1[s,:]^T O2[s,:]  (PE)  ->  trans part = sum_ij C[i,j]X[i,j]
where O1/O2 are one-hot tiles of the (u, v) tag sequences built with
iota+is_equal, and the diagonal/weighted sums use DVE tensor_tensor_reduce.
The host only sums the tiny per-core partial outputs (the unshard step).
"""

import numpy as np

B, S, T = 256, 1024, 128
H = S // 2              # 512 steps per core
NB = 64                 # batches per core
NCH = H // 128          # 4 chunks of 128 steps
C0 = float(np.log(float(T)) + 0.5)

_CACHE = {}


def _build_program(with_scores=True):
    import sys
    if "/opt/trn_rl_repo" not in sys.path:
        sys.path.insert(0, "/opt/trn_rl_repo")
    from contextlib import ExitStack

    import concourse.bass as bass
    import concourse.tile as tile
    from concourse import mybir

    f32 = mybir.dt.float32
    bf16 = mybir.dt.bfloat16
    i32 = mybir.dt.int32
    EQ = mybir.AluOpType.is_equal
    MUL = mybir.AluOpType.mult
    ADD = mybir.AluOpType.add
    EXP = mybir.ActivationFunctionType.Exp

    nc = bass.Bass("TRN2", target_bir_lowering=False)

    # register -C0 as a const AP (preamble, like Bass.__init__'s 0.0/1.0)
    _cb = nc.alloc_sbuf_tensor("const-negC0", [128, 1], f32)
    nc.gpsimd.memset(_cb.ap(), -C0)
    nc.const_aps.aps[(f32, -C0)] = _cb.ap()
    nc.all_engine_barrier()

    emis = nc.dram_tensor("emis", [NB, H, T], f32, kind="ExternalInput")
    transm = nc.dram_tensor("transm", [T, T], f32, kind="ExternalInput")
    bvec = nc.dram_tensor("bvec", [T, 1], f32, kind="ExternalInput")
    bvec_rep = nc.dram_tensor("bvec_rep", [NB, T], f32, kind="ExternalInput")
    tags_u = nc.dram_tensor("tags_u", [128, NB * NCH], f32, kind="ExternalInput")
    tags_v = nc.dram_tensor("tags_v", [128, NB * NCH], f32, kind="ExternalInput")
    tag0 = nc.dram_tensor("tag0", [NB, 1], f32, kind="ExternalInput")

    chi_o = nc.dram_tensor("chi", [T, NB], f32, kind="ExternalOutput")
    emitd_o = nc.dram_tensor("emitd", [T, NB], f32, kind="ExternalOutput")
    transd_o = nc.dram_tensor("transd", [T, NB], f32, kind="ExternalOutput")
    bterm_o = nc.dram_tensor("bterm", [NB, 1], f32, kind="ExternalOutput")

    with tile.TileContext(nc) as tc, ExitStack() as ctx:
        ep = lambda name, bufs, **kw: ctx.enter_context(
            tc.tile_pool(name=name, bufs=bufs, **kw))

        const_p = ep("const", 1)
        p1_f32 = ep("p1f32", 2)      # pass1 f32 slabs [128, 4096]
        p1_exp = ep("p1exp", 2)      # exp'ed bf16 slabs [128, 4096]
        expe_p = ep("expe", 1)       # big resident expE
        p2_f32 = ep("p2f32", 2)      # pass2 f32 slabs [128, 2048]
        e16_p = ep("e16", 9)         # raw-bf16 slabs for scores [128, 2048]
        o_p = ep("onehot", 12)       # one-hot tiles
        scr_p = ep("scratch", 4)     # TTR full-output scratch
        p_p = ep("pstate", 4)        # scan state tiles
        z_p = ep("zpsum", 2, space="PSUM")
        g_p = ep("gpsum", 2, space="PSUM")
        c_p = ep("cpsum", 2, space="PSUM")

        # ---------- constants / small inputs ----------
        trans_sb = const_p.tile([T, T], f32)
        nc.sync.dma_start(trans_sb[:], transm[:, :])
        bvec_sb = const_p.tile([T, 1], f32)
        nc.sync.dma_start(bvec_sb[:], bvec[:, :])
        bvb = const_p.tile([NB, T], f32)            # bvec broadcast on parts
        nc.sync.dma_start(bvb[:], bvec_rep[:, :])
        tu_sb = const_p.tile([128, NB * NCH], f32)
        nc.sync.dma_start(tu_sb[:], tags_u[:, :])
        tv_sb = const_p.tile([128, NB * NCH], f32)
        nc.sync.dma_start(tv_sb[:], tags_v[:, :])
        tag0_sb = const_p.tile([NB, 1], f32)
        nc.sync.dma_start(tag0_sb[:], tag0[:, :])

        iota_j = const_p.tile([128, 128], i32)      # value = free index j
        nc.gpsimd.iota(iota_j[:], [[1, 128]], channel_multiplier=0)
        # bf16 copy (values 0..127 exact): 2-byte in0 lets the EQ
        # tensor_scalars hit the DVE 2x_1p perf mode.
        iota_b = const_p.tile([128, 128], bf16)
        nc.vector.tensor_copy(iota_b[:], iota_j[:])
        iota_d = const_p.tile([128, 128], i32)      # value = j - p
        nc.gpsimd.iota(iota_d[:], [[1, 128]], channel_multiplier=-1)
        ident = const_p.tile([128, 128], bf16)
        nc.vector.tensor_scalar(out=ident[:], in0=iota_d[:], scalar1=0.0,
                                scalar2=None, op0=EQ)

        expT = const_p.tile([T, T], bf16)           # scan stationary
        nc.scalar.activation(expT[:], trans_sb[:], EXP)
        expb = const_p.tile([T, 1], f32)            # exp(boundary vec)
        nc.scalar.activation(expb[:], bvec_sb[:], EXP)

        # "warm" engines on freshly-DMA'ed tiles with plain copies so that
        # later Ptr-variant instructions (tensor_scalar with AP scalar) don't
        # need multi-sem waits (their ISA encoding has fewer wait slots).
        wrm = const_p.tile([128, 8], f32)
        nc.gpsimd.tensor_copy(wrm[:, 0:1], tu_sb[:, 0:1])
        nc.vector.tensor_copy(wrm[:, 1:2], tv_sb[:, 0:1])
        nc.vector.tensor_copy(wrm[:, 2:3], trans_sb[:, 0:1])
        nc.vector.tensor_copy(wrm[:, 3:4], expb[:])
        nc.vector.tensor_copy(wrm[0:NB, 4:5], tag0_sb[:])
        nc.vector.tensor_copy(wrm[0:NB, 5:6], bvb[:, 0:1])

        # boundary term: bterm[b] = bvec[tag0_b]
        o0 = const_p.tile([NB, 128], bf16)
        nc.vector.tensor_scalar(out=o0[:], in0=iota_j[0:NB, :],
                                scalar1=tag0_sb[:], scalar2=None, op0=EQ)
        bterm_sb = const_p.tile([NB, 1], f32)
        scr0 = const_p.tile([NB, 128], f32)
        nc.vector.tensor_tensor(out=scr0[:], in0=o0[:], in1=bvb[:], op=MUL)
        nc.vector.tensor_reduce(out=bterm_sb[:], in_=scr0[:],
                                axis=mybir.AxisListType.X, op=ADD)
        nc.sync.dma_start(bterm_o[:, :], bterm_sb[:])

        # ---------- pass 1: emissions -> expE[j, b*512 + t] (bf16) ----------
        # Phased so the scan (gated on chunk 0 of ALL batches) starts after
        # only 4MB of input: phase chunks (0,), then (1,2), then (3,).
        expE = expe_p.tile([128, NB * H], bf16)
        for c0, nch in ((0, 1), (1, 2), (3, 1)):
            emis_ph = emis[:, c0 * 128:(c0 + nch) * 128, :].rearrange(
                "(bg b) (c p) j -> bg p b c j", bg=4, b=16, c=nch, p=128)
            w = nch * 128
            for bg in range(4):
                slab = p1_f32.tile([128, 16 * w], f32, tag="p1slab")
                slab4 = slab[:].rearrange("p (b c j) -> p b c j", b=16, c=nch,
                                          j=128)
                for c in range(nch):
                    nc.scalar.dma_start(slab4[:, :, c, :],
                                        emis_ph[bg][:, :, c, :])
                eslab = p1_exp.tile([128, 16 * w], bf16, tag="p1exp")
                nc.scalar.activation(eslab[:], slab[:], EXP, bias=-C0)
                for b in range(16):
                    gb = bg * 16 + b
                    dst = expE[:, gb * H + c0 * 128: gb * H + c0 * 128 + w]
                    if nch > 1:
                        dst = dst.rearrange("p (c s) -> p c s", c=nch)
                    eng = nc.sync if c0 == 0 else nc.scalar
                    eng.dma_start_transpose(dst, eslab[:, b * w:(b + 1) * w])

        expE_t = expE[:].rearrange("p (b s) -> p s b", b=NB)  # [128, 512, 64]

        # ---------- pass 2 loads (for scores) + score blocks ----------
        emis_p2 = emis[:, :, :].rearrange(
            "(bg b) (c p) j -> bg c p b j", bg=4, b=16, c=4, p=128)
        e16 = {}

        def load_pass2(bg):
            def fn():
                for c in range(NCH):
                    slab = p2_f32.tile([128, 2048], f32)
                    nc.sync.dma_start(slab[:], emis_p2[bg, c])
                    t16 = e16_p.tile([128, 2048], bf16)
                    nc.gpsimd.tensor_copy(t16[:], slab[:])
                    e16[(bg, c)] = t16
            return fn

        def score_block(b):
            # split into two events (G part, C part) so each PE burst that
            # lands between serial scan matmuls stays short
            bg, bi = divmod(b, 16)
            o1s, o2s = [], []

            def fn_g():
                for c in range(NCH):
                    col = b * NCH + c
                    o1 = o_p.tile([128, 128], bf16, tag="o1")
                    nc.gpsimd.tensor_scalar(out=o1[:], in0=iota_b[:],
                                            scalar1=tu_sb[:, col:col + 1],
                                            scalar2=None, op0=EQ)
                    o1s.append(o1)
                g_ps = g_p.tile([128, 128], mybir.dt.float32, space="PSUM")
                for c in range(NCH):
                    nc.tensor.matmul(out=g_ps[:],
                                     lhsT=e16[(bg, c)][:, bi * 128:(bi + 1) * 128],
                                     rhs=o1s[c][:],
                                     start=(c == 0), stop=(c == NCH - 1))
                scr = scr_p.tile([128, 128], f32, tag="sg")
                nc.vector.tensor_tensor(out=scr[:], in0=g_ps[:], in1=ident[:],
                                        op=MUL)
                nc.vector.tensor_reduce(out=emitd_sb[:, b:b + 1], in_=scr[:],
                                        axis=mybir.AxisListType.X, op=ADD)

            def fn_c():
                for c in range(NCH):
                    col = b * NCH + c
                    o2 = o_p.tile([128, 128], bf16, tag="o2")
                    nc.vector.tensor_scalar(out=o2[:], in0=iota_b[:],
                                            scalar1=tv_sb[:, col:col + 1],
                                            scalar2=None, op0=EQ)
                    o2s.append(o2)
                c_ps = c_p.tile([128, 128], mybir.dt.float32, space="PSUM")
                for c in range(NCH):
                    nc.tensor.matmul(out=c_ps[:], lhsT=o1s[c][:], rhs=o2s[c][:],
                                     start=(c == 0), stop=(c == NCH - 1))
                scr2 = scr_p.tile([128, 128], f32, tag="sc")
                nc.vector.tensor_tensor(out=scr2[:], in0=c_ps[:],
                                        in1=trans_sb[:], op=MUL)
                nc.vector.tensor_reduce(out=transd_sb[:, b:b + 1], in_=scr2[:],
                                        axis=mybir.AxisListType.X, op=ADD)
            return fn_g, fn_c

        emitd_sb = const_p.tile([T, NB], f32)
        transd_sb = const_p.tile([T, NB], f32)

        events = {}
        if with_scores:
            events.setdefault(1, []).append(load_pass2(0))
            for bg in range(1, 4):
                events.setdefault(128 * bg - 60, []).append(load_pass2(bg))
            for b in range(NB):
                bg, bi = divmod(b, 16)
                fn_g, fn_c = score_block(b)
                def both(g=fn_g, c=fn_c):
                    g(); c()
                events.setdefault(128 * bg + 10 + 7 * bi, []).append(both)

        # ---------- the scan ----------
        wrmE = const_p.tile([128, NB], bf16)
        nc.vector.tensor_copy(wrmE[:], expE_t[:, 0])
        P = p_p.tile([128, NB], bf16)
        nc.vector.tensor_tensor(out=P[:], in0=expE_t[:, 0],
                                in1=expb[:].to_broadcast([128, NB]), op=MUL)
        for t in range(1, H):
            z = z_p.tile([128, NB], mybir.dt.float32, space="PSUM")
            nc.tensor.matmul(out=z[:], lhsT=expT[:], rhs=P[:],
                             start=True, stop=True)
            Pn = p_p.tile([128, NB], bf16)
            nc.vector.tensor_tensor(out=Pn[:], in0=z[:], in1=expE_t[:, t],
                                    op=MUL)
            P = Pn
            for fn in events.get(t, ()):
                fn()

        chi_ps = z_p.tile([128, NB], mybir.dt.float32, space="PSUM")
        nc.tensor.matmul(out=chi_ps[:], lhsT=expT[:], rhs=P[:],
                         start=True, stop=True)
        chi_sb = const_p.tile([T, NB], f32)
        nc.scalar.copy(chi_sb[:], chi_ps[:])
        nc.sync.dma_start(chi_o[:, :], chi_sb[:])
        if with_scores:
            nc.sync.dma_start(emitd_o[:, :], emitd_sb[:])
            nc.sync.dma_start(transd_o[:, :], transd_sb[:])

    _split_excess_waits(nc)
    _ldw_dedup(nc)
    return nc


def _ldw_dedup(nc):
    """Remove InstLdweights whose weights AP matches the previous weight
    load on the PE stream (only matmuls in between, no sync side-effects):
    the PE array already holds those weights. Saves ~7/8 of the scan's
    1024 weight reloads of the constant exp(transitions) stationary."""
    n = 0
    for f in nc.m.functions:
        for b in f.blocks:
            insts = list(b.instructions)
            out = []
            prev_w = None
            for ins in insts:
                tn = type(ins).__name__
                if tn == "InstLdweights":
                    key = str(ins.ins)
                    si = ins.sync_info
                    if key == prev_w and not (si and (si.on_wait or si.on_update)):
                        n += 1
                        continue
                    prev_w = key
                out.append(ins)
            if n:
                b.instructions = out
    return n


def _split_excess_waits(nc, limit=1):
    """This container's pinned walrus rejects instructions with more sync
    waits than the ISA encoding holds ("Too many sync wait commands").
    Hoist excess waits onto same-engine NoOps placed just before."""
    import bass_rust
    from concourse import mybir
    n = 0
    for f in nc.m.functions:
        for b in f.blocks:
            insts = list(b.instructions)
            out = []
            changed = False
            for ins in insts:
                si = ins.sync_info
                if si is not None and si.on_wait and len(si.on_wait) > limit:
                    w = list(si.on_wait)
                    extra, keep = w[:-limit], w[-limit:]
                    for i in range(0, len(extra), limit):
                        n += 1
                        out.append(mybir.InstNoOp(
                            name=f"{ins.name}-wsplit{i}", engine=ins.engine,
                            ins=[], outs=[],
                            sync_info=bass_rust.SyncInfo(
                                on_wait=extra[i:i + limit], on_update=[])))
                    ins.sync_info = bass_rust.SyncInfo(
                        on_wait=keep, on_update=list(si.on_update or []))
                    changed = True
                out.append(ins)
            if changed:
                b.instructions = out
    return n


def _numpy_fallback(emissions, tags, mask, transitions, start_transitions,
                    end_transitions):
    """Reference-equivalent numpy path (only used when mask isn't all ones)."""
    e = emissions.astype(np.float64)
    A = transitions.astype(np.float64)
    st = start_transitions.astype(np.float64)
    en = end_transitions.astype(np.float64)
    tg = tags.astype(np.int64)
    m = mask.astype(bool)
    mf = m.astype(np.float64)
    Bn, Sn, Tn = e.shape
    emit_sc = np.take_along_axis(e, tg[:, :, None], axis=2)[:, :, 0]
    first = st[tg[:, 0]] + emit_sc[:, 0]
    trans_sc = A[tg[:, :-1], tg[:, 1:]]
    steps = (trans_sc + emit_sc[:, 1:]) * mf[:, 1:]
    last_pos = m.astype(np.int64).sum(1) - 1
    last_tags = np.take_along_axis(tg, last_pos[:, None], axis=1)[:, 0]
    score = first + steps.sum(1) + en[last_tags]
    alpha = st[None, :] + e[:, 0]
    for t in range(1, Sn):
        x = alpha[:, :, None] + A[None, :, :] + e[:, t][:, None, :]
        mx = x.max(axis=1, keepdims=True)
        na = np.log(np.exp(x - mx).sum(axis=1)) + mx[:, 0, :]
        alpha = np.where(m[:, t][:, None], na, alpha)
    mx = (alpha + en[None, :]).max(axis=1)
    logz = np.log(np.exp(alpha + en[None, :] - mx[:, None]).sum(1)) + mx
    return np.float32((logz - score).mean())


def kernel(emissions, tags, mask, transitions, start_transitions,
           end_transitions):
    emissions = np.asarray(emissions, dtype=np.float32)
    tags = np.asarray(tags).astype(np.int64)
    mask = np.asarray(mask).astype(bool)
    transitions = np.asarray(transitions, dtype=np.float32)
    start_transitions = np.asarray(start_transitions, dtype=np.float32)
    end_transitions = np.asarray(end_transitions, dtype=np.float32)

    if not mask.all():
        return _numpy_fallback(emissions, tags, mask, transitions,
                               start_transitions, end_transitions)

    import sys
    if "/opt/trn_rl_repo" not in sys.path:
        sys.path.insert(0, "/opt/trn_rl_repo")
    from concourse.bass_utils import run_bass_kernel_spmd

    if "nc" not in _CACHE:
        _CACHE["nc"] = _build_program()
    nc = _CACHE["nc"]

    def tags_dev(u):
        # [64, 512] -> [128, 256] with col = b*4 + c
        return np.ascontiguousarray(
            u.reshape(NB, NCH, 128).transpose(2, 0, 1).reshape(128, NB * NCH)
            .astype(np.float32))

    in_maps = []
    for core in range(8):
        g, d = divmod(core, 2)
        bs = slice(NB * g, NB * g + NB)
        tl = tags[bs]
        if d == 0:  # forward
            em = emissions[bs, :H]
            u = tl[:, 0:H]
            v = tl[:, 1:H + 1]
            t0 = tl[:, 0]
            X = transitions
            bv = start_transitions
        else:       # backward (time-reversed)
            em = emissions[bs, ::-1][:, :H]
            tlr = tl[:, ::-1]
            u = tlr[:, 0:H].copy()
            v = tlr[:, 1:H + 1].copy()
            u[:, H - 1] = 0
            v[:, H - 1] = 0
            t0 = tlr[:, 0]
            X = np.ascontiguousarray(transitions.T)
            bv = end_transitions
        in_maps.append({
            "emis": np.ascontiguousarray(em, dtype=np.float32),
            "transm": np.ascontiguousarray(X, dtype=np.float32),
            "bvec": bv.reshape(T, 1).astype(np.float32),
            "bvec_rep": np.ascontiguousarray(
                np.broadcast_to(bv.reshape(1, T), (NB, T)), dtype=np.float32),
            "tags_u": tags_dev(u),
            "tags_v": tags_dev(v),
            "tag0": t0.reshape(NB, 1).astype(np.float32),
        })

    import os
    trace = bool(int(os.environ.get("CRF_TRACE", "0")))
    _CACHE["last_in_maps"] = in_maps
    res = run_bass_kernel_spmd(nc, in_maps, core_ids=list(range(8)),
                               trace=trace)
    _CACHE["last_results"] = res
    outs = res.results

    logz = np.zeros(B, np.float64)
    score = np.zeros(B, np.float64)
    corr = float(transitions[0, 0])
    for g in range(4):
        f, bw = outs[2 * g], outs[2 * g + 1]
        bs = slice(NB * g, NB * g + NB)
        prod = (f["chi"].astype(np.float64) * bw["chi"].astype(np.float64))
        logz[bs] = np.log(prod.sum(axis=0)) + S * C0
        sc = (f["emitd"] + bw["emitd"] + f["transd"] + bw["transd"]).astype(
            np.float64).sum(axis=0)
        score[bs] = sc + f["bterm"][:, 0] + bw["bterm"][:, 0] - corr
    return np.float32((logz - score).mean())

